# revision 1
# baseline (speedup 1.0000x reference)
"""MEX (log-sum-exp) 3x3 pooling kernel for Trainium2, 8-core SPMD.

Math: out[b,m,i,j] = log( (1/n) * sum_{c,dh,dw} exp(x[b,c,i+dh,j+dw] + off[m,c,dh,dw]) )
with n = C*3*3 = 576, eps = 1.

Identity used: the reference's per-pixel max-stabilization cancels exactly:
  out = m_x + m_b + log(S) - log(n)  ==  log( sum_k exp(x_k + b_k) ) - log(n)
Values are benign in fp32 (x ~ N(0,1) -> exp(x) in [4e-3, 260]; off = log_softmax
values in [-13, -2.5] -> exp(off) in [2e-6, 0.08]), so no stabilization is needed.

Per-core plan (core i handles batch images 2i, 2i+1):
  - SBUF E tile (128, 16384+pad): partition p = img*64 + c, free = h*128 + w,
    E = exp(x) computed by ACT in chunks as DMA streams x in.
  - Weights: off permuted host-side to wp[c, (dh,dw,m)] (64,144); device computes
    exp and scatters into lhsT layout LT[img*64+c, dh*96 + dw*32 + img*16 + m],
    zero elsewhere (block-diagonal over img so the two images stay independent).
  - For each 2048-pixel superchunk: 3 PSUM-accumulated matmuls (over dh, each
    N=512 x 4 psum banks) with rhs = E slice at offset dh*128.  Output psum
    P[(dw,img,m), n] holds per-dw-tap partial sums.
  - Two DVE shifted adds combine the dw groups:
      OUT[img*16+m, j] = P[p,j] + P[32+p, j+1] + P[64+p, j+2]
  - ACT computes log(OUT * 1/576) and a strided DMA writes the valid
    (row, col<=125) region to DRAM.
"""

import numpy as np

EPS = 1.0
B, C, H, W = 16, 64, 128, 128
M = 16
BH = BW = 3
HO, WO = H - BH + 1, W - BW + 1  # 126, 126
N_TAPS = C * BH * BW  # 576
NCORES = 8
BPC = B // NCORES  # 2 images per core
HWP = H * W  # 16384 pixels per image plane
PAD = 768
SC = 2048  # superchunk pixels (4 psum banks of 512 fp32)
NSC = HWP // SC  # 8
DMACH = 1024  # x DMA / exp chunk size in pixels
NDMACH = HWP // DMACH

# matmul dtype: "f32r" = single-pass fp32 (full rate, slightly reduced HW
# precision), "f32" = exact fp32 (4 cycles/row).
import os as _os

MM_DTYPE = _os.environ.get("MEX_MM_DTYPE", "f16")
# fp16 weight pre-scale: keeps exp(offsets) (values down to 2e-6) inside the
# fp16 normal range; cancels exactly in the final log's input scale.
W_ALPHA_LOG = {"f16": 10.0 * 0.6931471805599453}.get(MM_DTYPE, 0.0)

_BUILT = {}


def _build(mm_dtype: str):
    """Build (and cache) the Bass/Tile program shared by all 8 cores."""
    if mm_dtype in _BUILT:
        return _BUILT[mm_dtype]

    import concourse.bass as bass
    import concourse.bacc as bacc
    import concourse.tile as tile
    from concourse import mybir

    f32 = mybir.dt.float32
    # dtype of matmul operand tiles: walrus requires producers of f32r matmul
    # operands to emit f32r-typed outputs, so E/QE/LT are natively f32r.
    mdt = {
        "f32r": mybir.dt.float32r,
        "f32": f32,
        "bf16": mybir.dt.bfloat16,
        "f16": mybir.dt.float16,
    }[mm_dtype]
    mdt_small = mdt in (mybir.dt.bfloat16, mybir.dt.float16)
    import math as _math

    w_alpha_log = 10.0 * _math.log(2.0) if mm_dtype == "f16" else 0.0
    AF = mybir.ActivationFunctionType

    nc = bacc.Bacc("TRN2", target_bir_lowering=False, debug=False)

    xd = nc.dram_tensor("x", [128, HWP], f32, kind="ExternalInput")
    wpd = nc.dram_tensor("wp", [64, 144], f32, kind="ExternalInput")
    outd = nc.dram_tensor("out", [BPC * M, HO * WO], f32, kind="ExternalOutput")

    with tile.TileContext(nc) as tc:
        with (
            tc.tile_pool(name="singles", bufs=1) as singles,
            tc.tile_pool(name="xin", bufs=10 if mdt_small else 5) as xin,
            tc.tile_pool(name="psum", bufs=2, space="PSUM") as psum,
            tc.tile_pool(name="post", bufs=2) as post,
        ):
            # ---- weights DMA first (the weights chain must not wait on the
            # x stream: DMA-completion sems are cumulative, so wp must tick
            # before the 16 x transfers), then all x DMA triggers so the
            # transfers overlap the weights chain. ----
            Q = singles.tile([128, 144], f32)
            nc.sync.dma_start(out=Q[0:64, :], in_=bass.AP(wpd, 0, [[144, 64], [1, 144]]))
            nc.sync.dma_start(out=Q[64:128, :], in_=bass.AP(wpd, 0, [[144, 64], [1, 144]]))
            # ---- weights: wp -> scatter into pre-exp lhsT layout -> exp ----
            # (memset is not legal on f32r/f16 matmul-operand tiles here, so
            # build an f32 "log-domain" LT with -80 in the zero slots and exp
            # through ACT: exp(-80) ~ 2e-35 is negligible vs S>=1e-3.)
            QL = singles.tile([128, 288], f32)
            nc.vector.memset(QL[:, :], -80.0)
            QLv = QL[:, :].rearrange("p (dh dw i m) -> p dh dw i m", dh=3, dw=3, i=2)
            Qv = Q[:, :].rearrange("p (dh dw m) -> p dh dw m", dh=3, dw=3)
            nc.vector.tensor_scalar_add(
                out=QLv[0:64, :, :, 0, :], in0=Qv[0:64], scalar1=w_alpha_log
            )
            nc.vector.tensor_scalar_add(
                out=QLv[64:128, :, :, 1, :], in0=Qv[64:128], scalar1=w_alpha_log
            )
            LT = singles.tile([128, 288], mdt)
            nc.scalar.activation(out=LT[:, :], in_=QL[:, :], func=AF.Exp, scale=EPS)

            # ---- E = exp(x), streamed; pad gets exp(0)=1 ----
            E = singles.tile([128, HWP + PAD], mdt)
            Xpad = singles.tile([128, PAD], f32)
            nc.vector.memset(Xpad[:, :], 0.0)
            nc.scalar.activation(
                out=E[:, HWP:], in_=Xpad[:, :], func=AF.Exp, scale=EPS
            )
            # small leading chunks so the first exp (and so the first
            # matmul) starts as early as possible.
            xchunks = [512, 512, 1024] + [2048] * 7
            xoff = 0
            for npx in xchunks:
                Xk = xin.tile([128, npx], f32, tag="Xk")
                nc.sync.dma_start(
                    out=Xk[:, :],
                    in_=bass.AP(xd, xoff, [[HWP, 128], [1, npx]]),
                )
                nc.scalar.activation(
                    out=E[:, xoff : xoff + npx],
                    in_=Xk[:, :],
                    func=AF.Exp,
                    scale=EPS,
                )
                xoff += npx

            # ---- main conv + log loop (software-pipelined emission) ----
            # Per superchunk: 12 matmuls -> evacuate the dw=1 group to SBUF
            # (copy; DVE while DVE is otherwise idle early, ACT for the later
            # superchunks to balance totals) -> add1 = P[0:32](psum)+Pc(sbuf)
            # -> add2 = A(sbuf)+P[64:96](psum) (mixed-space operands may have
            # different base partitions; both-SBUF may not) -> ACT log ->
            # strided DMA out.  Superchunk s's copy is emitted before s-1's
            # adds so every engine has ready work one stage ahead.
            LTd = LT[:, :].rearrange("p (dh c) -> p dh c", dh=3)
            ln_scale = 1.0 / (float(N_TAPS) * _math.exp(w_alpha_log))
            # chunk list: big superchunks for the body, smaller ones at the
            # tail so the final copy->add->add->log->dma chain is short.
            chunks = [(i * SC, SC) for i in range(6)] + [
                (6 * SC + i * 1024, 1024) for i in range(4)
            ]
            # dw=1 evacuation goes to ACT only on the late chunks (after the
            # exp stream has drained); DVE handles it early.
            ACT_COPY_CS = (5, 6, 7, 8, 9)
            pend = None  # (P_tile, Pc_tile, p0, npix) waiting for finish

            def _finish(pend):
                Pv, Pcv, p0, npix = pend
                nv = npix - 2
                A = post.tile([32, nv], f32, tag="A")
                nc.vector.tensor_add(out=A[:, :], in0=Pv[0:32, 0:nv], in1=Pcv[:, :])
                S2 = post.tile([32, nv], f32, tag="S2")
                nc.vector.tensor_add(out=S2[:, :], in0=A[:, :], in1=Pv[64:96, 2:npix])
                LG = post.tile([32, nv], f32, tag="LG")
                nc.scalar.activation(
                    out=LG[:, :], in_=S2[:, :], func=AF.Ln, scale=ln_scale
                )
                row0 = p0 // W
                nrows = min(npix // W, HO - row0)
                lg = LG[:, :]
                src = bass.AP(lg.tensor, lg.offset, [lg.ap[0], [W, nrows], [1, WO]])
                dst = bass.AP(
                    outd, row0 * WO, [[HO * WO, BPC * M], [WO, nrows], [1, WO]]
                )
                nc.sync.dma_start(out=dst, in_=src)

            for cs, (p0, npix) in enumerate(chunks):
                P = psum.tile([96, npix], f32, tag="P")
                for dh in range(3):
                    lhsT = LTd[:, dh, :]
                    for b4 in range(npix // 512):
                        base = p0 + dh * W + b4 * 512
                        nc.tensor.matmul(
                            P[:, b4 * 512 : (b4 + 1) * 512],
                            lhsT,
                            E[:, base : base + 512],
                            start=(dh == 0),
                            stop=(dh == 2),
                        )
                nv = npix - 2
                Pc = post.tile([32, nv], f32, tag="Pc")
                if cs in ACT_COPY_CS:
                    nc.scalar.copy(out=Pc[:, :], in_=P[32:64, 1 : npix - 1])
                else:
                    nc.vector.tensor_copy(out=Pc[:, :], in_=P[32:64, 1 : npix - 1])
                if pend is not None:
                    _finish(pend)
                pend = (P, Pc, p0, npix)
            _finish(pend)

    nc.compile()
    _BUILT[mm_dtype] = nc
    return nc


def _prep_inputs(x, offsets):
    x = np.ascontiguousarray(np.asarray(x), dtype=np.float32)
    off = np.asarray(offsets, dtype=np.float32).reshape(M, C, BH, BW)
    # wp[c, dh*48 + dw*16 + m] = off[m, c, dh, dw]
    wp = np.ascontiguousarray(np.transpose(off, (1, 2, 3, 0)).reshape(64, 144))
    in_maps = [
        {"x": np.ascontiguousarray(x[BPC * i : BPC * (i + 1)]).reshape(128, HWP), "wp": wp}
        for i in range(NCORES)
    ]
    return in_maps


def kernel(x, offsets):
    from concourse.bass_utils import run_bass_kernel_spmd

    nc = _build(MM_DTYPE)
    in_maps = _prep_inputs(x, offsets)
    res = run_bass_kernel_spmd(nc, in_maps, core_ids=list(range(NCORES)))
    out = np.empty((B, M, HO, WO), dtype=np.float32)
    for i in range(NCORES):
        out[BPC * i : BPC * (i + 1)] = res.results[i]["out"].reshape(BPC, M, HO, WO)
    return out



# revision 2
# speedup vs baseline: 1.0962x; 1.0962x over previous
"""MEX (log-sum-exp) 3x3 pooling kernel for Trainium2, 8-core SPMD.

Math: out[b,m,i,j] = log( (1/n) * sum_{c,dh,dw} exp(x[b,c,i+dh,j+dw] + off[m,c,dh,dw]) )
with n = C*3*3 = 576, eps = 1.

Identity used: the reference's per-pixel max-stabilization cancels exactly:
  out = m_x + m_b + log(S) - log(n)  ==  log( sum_k exp(x_k + b_k) ) - log(n)
Values are benign in fp32, no stabilization needed (see baseline notes).

Per-core plan (core i handles batch images 2i, 2i+1):
  - E[p = img*64+c, h*128+w] = exp(x) fp16, streamed in chunks (ACT).
  - LT[p, dh*96 + dw*32 + img*16 + m] = exp(off + alpha) fp16, block-diagonal
    over img (weights chain as in the baseline).
  - Superchunk s (2048 px = 16 output rows): 12 PSUM-accumulated matmuls
    (3 dh x 4 banks of 512) -> P[(dw,img,m), pix] partial sums per dw tap.
  - Evacuate: ONE copy P[96,2048] -> CW bf16 in SBUF (DVE for even s, ACT for
    odd s).  Cost is free-dim-bound, so copying all 96 rows costs the same as
    32 -- this replaces the baseline's copy+2 psum adds at 32 lanes.
  - Fold (per wave of 2 superchunks = 4096 px): 12 SBUF->SBUF DMAs remap the
    three 32-row dw groups into F0/F1/F2[128 = 4x32 rows, 1024] with the dw
    column shifts baked into the src offsets.  DMA is the only engine that
    can move data across partitions.
  - Combine: S = F0+F1+F2 via two DVE bf16 adds at 128 partitions (2x mode),
    4x less free-dim than the baseline's 32-partition psum adds.
  - ACT Ln on [128, 1024] (4x less FD than 32-partition log), strided DMA to
    a permuted DRAM layout; the host decodes it (allowed: host only reshapes).
"""

import math as _math
import os as _os

import numpy as np

EPS = 1.0
B, C, H, W = 16, 64, 128, 128
M = 16
BH = BW = 3
HO, WO = H - BH + 1, W - BW + 1  # 126, 126
N_TAPS = C * BH * BW  # 576
NCORES = 8
BPC = B // NCORES  # 2 images per core
HWP = H * W  # 16384 pixels per image plane
PAD = 768
SC = 2048  # superchunk pixels (4 psum banks of 512 fp32)
NSC = HWP // SC  # 8
WAVE = 2 * SC  # fold wave = 2 superchunks = 4096 px = 32 output rows
NWAVE = HWP // WAVE  # 4
FW = WAVE // 4  # fold window = 1024 px = 8 output rows per 32-partition group

MM_DTYPE = _os.environ.get("MEX_MM_DTYPE", "f16")
W_ALPHA_LOG = {"f16": 10.0 * 0.6931471805599453}.get(MM_DTYPE, 0.0)

_BUILT = {}


def _build(mm_dtype: str):
    """Build (and cache) the Bass/Tile program shared by all 8 cores."""
    if mm_dtype in _BUILT:
        return _BUILT[mm_dtype]

    import concourse.bass as bass
    import concourse.bacc as bacc
    import concourse.tile as tile
    from concourse import mybir

    f32 = mybir.dt.float32
    bf16 = mybir.dt.bfloat16
    mdt = {
        "f32r": mybir.dt.float32r,
        "f32": f32,
        "bf16": mybir.dt.bfloat16,
        "f16": mybir.dt.float16,
    }[mm_dtype]
    w_alpha_log = 10.0 * _math.log(2.0) if mm_dtype == "f16" else 0.0
    AF = mybir.ActivationFunctionType

    nc = bacc.Bacc("TRN2", target_bir_lowering=False, debug=False)

    xd = nc.dram_tensor("x", [128, HWP], f32, kind="ExternalInput")
    wpd = nc.dram_tensor("wp", [64, 144], f32, kind="ExternalInput")
    outd = nc.dram_tensor("out", [128, HWP // 4], f32, kind="ExternalOutput")

    with tile.TileContext(nc) as tc:
        with (
            tc.tile_pool(name="singles", bufs=1) as singles,
            tc.tile_pool(name="xin", bufs=10) as xin,
            tc.tile_pool(name="psum", bufs=2, space="PSUM") as psum,
            tc.tile_pool(name="post", bufs=2) as post,
        ):
            # ---- weights DMA first (must tick the DMA-completion sem before
            # the x stream), then the x DMAs so transfers overlap. ----
            Q = singles.tile([128, 144], f32)
            nc.sync.dma_start(out=Q[0:64, :], in_=bass.AP(wpd, 0, [[144, 64], [1, 144]]))
            nc.sync.dma_start(out=Q[64:128, :], in_=bass.AP(wpd, 0, [[144, 64], [1, 144]]))
            QL = singles.tile([128, 288], f32)
            nc.vector.memset(QL[:, :], -80.0)
            QLv = QL[:, :].rearrange("p (dh dw i m) -> p dh dw i m", dh=3, dw=3, i=2)
            Qv = Q[:, :].rearrange("p (dh dw m) -> p dh dw m", dh=3, dw=3)
            nc.vector.tensor_scalar_add(
                out=QLv[0:64, :, :, 0, :], in0=Qv[0:64], scalar1=w_alpha_log
            )
            nc.vector.tensor_scalar_add(
                out=QLv[64:128, :, :, 1, :], in0=Qv[64:128], scalar1=w_alpha_log
            )
            LT = singles.tile([128, 288], mdt)
            nc.scalar.activation(out=LT[:, :], in_=QL[:, :], func=AF.Exp, scale=EPS)

            # ---- E = exp(x); pad gets exp(0)=1 ----
            E = singles.tile([128, HWP + PAD], mdt)
            Xpad = singles.tile([128, PAD], f32)
            nc.vector.memset(Xpad[:, :], 0.0)
            nc.scalar.activation(out=E[:, HWP:], in_=Xpad[:, :], func=AF.Exp, scale=EPS)

            # wave buffers for the evacuated psum (bf16), padded so the fold
            # DMAs' +1/+2 column shifts stay in bounds (cols 4096..4103).
            CW0 = singles.tile([128, WAVE + 8], bf16)
            CW1 = singles.tile([128, WAVE + 8], bf16)
            CWT = [CW0, CW1]
            nc.vector.memset(CW0[:, WAVE : WAVE + 8], 0.0)
            nc.vector.memset(CW1[:, WAVE : WAVE + 8], 0.0)

            # x DMA all upfront; exp emission is interleaved into the main
            # loop below so ACT alternates exp / copies / Ln without stalls.
            xchunks = [512, 512, 1024] + [2048] * 7
            xoff = 0
            xk_tiles = []
            for npx in xchunks:
                Xk = xin.tile([128, npx], f32, tag="Xk")
                nc.sync.dma_start(
                    out=Xk[:, :], in_=bass.AP(xd, xoff, [[HWP, 128], [1, npx]])
                )
                xk_tiles.append((Xk, xoff, npx))
                xoff += npx

            def emit_exp(k):
                Xk, xo, npx = xk_tiles[k]
                nc.scalar.activation(
                    out=E[:, xo : xo + npx], in_=Xk[:, :], func=AF.Exp, scale=EPS
                )

            # exp for the first chunks so superchunk 0's matmuls can start.
            emit_exp(0)
            emit_exp(1)
            emit_exp(2)  # covers up to px 2048
            emit_exp(3)  # covers up to px 4096
            next_exp = 4

            LTd = LT[:, :].rearrange("p (dh c) -> p dh c", dh=3)
            ln_scale = 1.0 / (float(N_TAPS) * _math.exp(w_alpha_log))

            def emit_mains(s):
                P = psum.tile([96, SC], f32, tag="P")
                p0 = s * SC
                for dh in range(3):
                    lhsT = LTd[:, dh, :]
                    for b4 in range(SC // 512):
                        base = p0 + dh * W + b4 * 512
                        nc.tensor.matmul(
                            P[:, b4 * 512 : (b4 + 1) * 512],
                            lhsT,
                            E[:, base : base + 512],
                            start=(dh == 0),
                            stop=(dh == 2),
                        )
                return P

            def emit_wave_post(w):
                """fold (12 sb->sb DMAs) + 2 bf16 adds + Ln + out DMA."""
                CW = CWT[w % 2]
                F0 = post.tile([128, FW], bf16, tag="F0")
                F1 = post.tile([128, FW], bf16, tag="F1")
                F2 = post.tile([128, FW], bf16, tag="F2")
                for g, Fg in enumerate((F0, F1, F2)):
                    for q in range(4):
                        nc.sync.dma_start(
                            out=Fg[32 * q : 32 * q + 32, :],
                            in_=CW[32 * g : 32 * g + 32, FW * q + g : FW * q + g + FW],
                        )
                A = post.tile([128, FW], bf16, tag="A")
                nc.vector.tensor_add(out=A[:, :], in0=F0[:, :], in1=F1[:, :])
                S = post.tile([128, FW], bf16, tag="S")
                nc.vector.tensor_add(out=S[:, :], in0=A[:, :], in1=F2[:, :])
                LG = post.tile([128, FW], f32, tag="LG")
                nc.scalar.activation(out=LG[:, :], in_=S[:, :], func=AF.Ln, scale=ln_scale)
                nc.sync.dma_start(
                    out=bass.AP(outd, w * FW, [[HWP // 4, 128], [1, FW]]),
                    in_=LG[:, :],
                )

            for s in range(NSC):
                P = emit_mains(s)
                # evacuate all 96 psum rows in one FD-bound copy
                CW = CWT[(s // 2) % 2]
                half = (s % 2) * SC
                dst = CW[0:96, half : half + SC]
                if s % 2 == 0:
                    nc.vector.tensor_copy(out=dst, in_=P[:, :])
                else:
                    nc.scalar.copy(out=dst, in_=P[:, :])
                if next_exp < len(xchunks):
                    emit_exp(next_exp)
                    next_exp += 1
                if s % 2 == 1:
                    emit_wave_post(s // 2)

    nc.compile()
    _BUILT[mm_dtype] = nc
    return nc


def _prep_inputs(x, offsets):
    x = np.ascontiguousarray(np.asarray(x), dtype=np.float32)
    off = np.asarray(offsets, dtype=np.float32).reshape(M, C, BH, BW)
    # wp[c, dh*48 + dw*16 + m] = off[m, c, dh, dw]
    wp = np.ascontiguousarray(np.transpose(off, (1, 2, 3, 0)).reshape(64, 144))
    in_maps = [
        {"x": np.ascontiguousarray(x[BPC * i : BPC * (i + 1)]).reshape(128, HWP), "wp": wp}
        for i in range(NCORES)
    ]
    return in_maps


def _decode(raw):
    """raw [128, 4096] per core -> [BPC, M, HO, WO].

    raw[32q+p, 1024w + 128r + c] = out[(img,m)=p, row 32w+8q+r, col c].
    """
    a = raw.reshape(4, 32, NWAVE, 8, 128)  # [q, p, w, r, c]
    a = a.transpose(1, 2, 0, 3, 4).reshape(32, 128, 128)  # [p, row, col]
    return a[:, :HO, :WO].reshape(BPC, M, HO, WO)


def kernel(x, offsets):
    from concourse.bass_utils import run_bass_kernel_spmd

    nc = _build(MM_DTYPE)
    in_maps = _prep_inputs(x, offsets)
    res = run_bass_kernel_spmd(nc, in_maps, core_ids=list(range(NCORES)))
    out = np.empty((B, M, HO, WO), dtype=np.float32)
    for i in range(NCORES):
        out[BPC * i : BPC * (i + 1)] = _decode(res.results[i]["out"])
    return out
